# revision 2
# baseline (speedup 1.0000x reference)
"""MiniRocket feature extractor on 8 Trainium2 NeuronCores, v2.

Design (per core, 4 batch rows x 6 dilations = 24 (d,b) "units"):
  - x is staged to HBM as fp16 (halves DMA bytes; conv error ~2.5e-3 rms on a
    resp scale of ~5, far inside the int16-quantization error budget).
  - One strided DMA per unit builds xshift [72, 4096] fp16 (9 dilated tap
    shifts x 8 channels of the zero-padded series).
  - Kernels are permuted parity-first per dilation and PACKED 128 rows per
    PSUM/count tile via quadrant-legal matmul positions:
      * 12 "full" tiles: even unit's 84 kernels (padded to 96) at rows 0:96
        plus odd unit's first 32 kernels at rows 96:128 (tile_position (0,96)).
      * 6 "piece" tiles: two odd units' kernels 32:84 (padded to 64) at rows
        0:64 and 64:128 (tile_position (0,64)).
    18 tiles instead of 24 cuts count passes 96 -> 72 and drains 48 -> 36.
  - ScalarE drains PSUM fp32 -> SBUF int16 with a per-row scale (all four
    bias thresholds sit inside +/-32000; round+saturate on conversion).
  - Trim-parity rows sit first in each row block, so one small strided memset
    writes -32600 sentinels into both pad-edge column ranges of those rows:
    the full-range count then equals the trimmed count (no edge passes, no
    B-coefficient in the finalize).
  - Counting: fused compare+count (is_gt + add-reduce, accum_out) per
    (tile, f): mostly VectorE (4x int16 mode), a few passes on GpSimd to
    balance; optional ScalarE sign-count offload.
  - Finalize on GpSimd: feat = count*A - C with per-row packed coefficients;
    features stage into one SBUF tile and leave in a single output DMA;
    the host inverts the packing permutation (pure numpy relabeling).
"""

import numpy as np
from contextlib import ExitStack

import concourse.bass as bass
import concourse.mybir as mybir
import concourse.tile as tile
from concourse.ap import AP
from concourse.bass_utils import run_bass_kernel_spmd

DILATIONS = (1, 2, 4, 8, 16, 32)
ND = 6
K = 84
KS = 9
C = 8
L = 4096
F = 4
B = 32
N_CORES = 8
B_LOC = 4
PADMAX = 128
LP = L + 2 * PADMAX
NFEAT = ND * K * F
NT = 18  # count tiles per core
SENTINEL = -30000.0

F32 = mybir.dt.float32
F16 = mybir.dt.float16
I16 = mybir.dt.int16

# Counting split: DVE runs f0..f2 full-range and most of f3; Act counts the
# first ACT_COLS of f3 via Sign+accum (GPSIMD cannot touch PSUM or run
# compare ops in this toolchain, so Pool only does memsets/finalize/DMA).
ACT_COLS = 384


def _perm(di):
    par = [k for k in range(K) if (di + k) % 2 == 1]
    even = [k for k in range(K) if (di + k) % 2 == 0]
    return par + even  # 42 + 42


def _tiles():
    """Static packing: list of (tile_kind, segments). Each segment is
    (unit, kernel_lo, kernel_hi, row_lo, n_pad_rows) in perm order; processing
    order is FT,FT,PT per group of 4 units."""
    out = []
    for j in range(ND):
        for i in (2 * j, 2 * j + 1):
            x, y = 2 * i, 2 * i + 1
            out.append(("FT", [(x, 0, 84, 0, 12), (y, 0, 32, 96, 0)]))
        y1, y2 = 4 * j + 1, 4 * j + 3
        out.append(("PT", [(y1, 32, 84, 0, 12), (y2, 32, 84, 64, 12)]))
    return out


TILES = _tiles()


def _split_excess_waits(nc, max_waits=1):
    """This walrus build allows only one sync-wait per instruction; hoist
    extra waits onto preceding NOPs of the same engine."""
    n = 0
    for f in nc.m.functions:
        for bb in f.blocks:
            insts = bb.instructions
            if not any(
                i.sync_info and i.sync_info.on_wait and len(i.sync_info.on_wait) > max_waits
                for i in insts
            ):
                continue
            out = []
            for inst in insts:
                si = inst.sync_info
                waits = list(si.on_wait) if si and si.on_wait else []
                if len(waits) > max_waits:
                    for w in waits[:-max_waits]:
                        nop = mybir.InstNoOp(name=f"syncfix-{n}", ins=[], outs=[])
                        n += 1
                        nop.engine = inst.engine
                        nop.sync_info = mybir.SyncInfo(on_wait=[w], on_update=[])
                        out.append(nop)
                    inst.sync_info = mybir.SyncInfo(
                        on_wait=waits[-max_waits:],
                        on_update=list(si.on_update or []),
                    )
                out.append(inst)
            bb.instructions = out


def _build_nc():
    nc = bass.Bass()
    xprep = nc.declare_dram_parameter("xprep", [B_LOC, C, LP], F16, isOutput=False)
    # per dilation: [0:96] full-unit weights, [96:128] first-32 piece,
    # [128:192] kernels 32:84 padded to 64
    wpack = nc.declare_dram_parameter("wpack", [ND, 72, 192], F16, isOutput=False)
    # per tile t: col 17t+0 scale, +1..4 bias16, +5..8 nbias16, +9..12 A, +13..16 C
    cpack = nc.declare_dram_parameter("cpack", [128, 16 * NT], F32, isOutput=False)
    out = nc.declare_dram_parameter("out", [128, 4 * NT], F32, isOutput=True)

    alu = mybir.AluOpType
    act = mybir.ActivationFunctionType

    with tile.TileContext(nc) as tc, ExitStack() as ctx:
        cpool = ctx.enter_context(tc.tile_pool(name="const", bufs=1))
        xsh_pool = ctx.enter_context(tc.tile_pool(name="xsh", bufs=8))
        psum_pool = ctx.enter_context(tc.tile_pool(name="psum", bufs=2, space="PSUM"))
        resp_pool = ctx.enter_context(tc.tile_pool(name="resp", bufs=3))
        trash_pool = ctx.enter_context(tc.tile_pool(name="trash", bufs=1))
        acc_pool = ctx.enter_context(tc.tile_pool(name="acc", bufs=6))
        fin_pool = ctx.enter_context(tc.tile_pool(name="fin", bufs=6))

        # xshift loads, one per unit, issued in first-use order with early
        # priority so the queue never starves compute. The first few issue
        # from different engines' DGE queues so their fixed issue costs
        # overlap instead of serializing on SP.
        xsh = {}

        def load_xsh(u, prio=None, eng=None):
            # one [72, 2048] tile per (unit, col-group): halves the transfer
            # the first drain waits on and smooths steady-state prefetch
            if (u, 0) in xsh:
                return
            di, b = divmod(u, 4)
            d = DILATIONS[di]
            pad = 4 * d
            for g in range(2):
                t_ = xsh_pool.tile([72, L // 2], F16)
                i_ = (eng or nc.sync).dma_start(
                    t_[:],
                    AP(
                        xprep,
                        b * C * LP + (PADMAX - pad) + g * 2048,
                        [[d, KS], [LP, C], [1, L // 2]],
                    ),
                )
                if prio is not None:
                    i_.ins.bass_priority = prio + g
                xsh[(u, g)] = t_

        # small consts first so the first drain isn't gated behind 1.2 MB of
        # xshift data; then the first two units' xshifts.
        w_t = cpool.tile([72, ND * 192], F16)
        _iw = nc.sync.dma_start(w_t[:], AP(wpack, 0, [[192, 72], [192 * 72, ND], [1, 192]]))
        _iw.ins.bass_priority = 1
        load_xsh(0, prio=2)
        load_xsh(1, prio=4, eng=nc.scalar)
        load_xsh(2, prio=6, eng=nc.gpsimd)
        load_xsh(3, prio=8)
        cp_t = cpool.tile([128, 16 * NT], F32, tag="cp_t")
        _ic = nc.sync.dma_start(cp_t[:], AP(cpack, 0, [[16 * NT, 128], [1, 16 * NT]]))
        _ic.ins.bass_priority = 10

        # prime the PE p-state while the first loads are in flight: ~3.5us of
        # dummy matmuls on a zeroed tile (no DMA dependency) so the first
        # real matmuls run at full clock. Uses a regular psum-pool buffer;
        # tile0's WAR on it resolves long before tile0's data lands.
        warm_src = cpool.tile([72, 512], F16, tag="warm_src")
        nc.vector.memset(warm_src[:], 0.0)
        ps_warm = psum_pool.tile([128, 2048], F32, tag="ps")
        for wi in range(8):
            nc.tensor.matmul(
                ps_warm[0:96, 0:512], warm_src[:, 0:96], warm_src[:], start=True, stop=True,
            )

        trash_d = trash_pool.tile([128, L], F16, tag="trash_d")
        trash_a = trash_pool.tile([128, ACT_COLS], F16, tag="trash_a")

        fn_global = cpool.tile([128, 4 * NT], F32, tag="fn_global")

        for t, (kind, segs) in enumerate(TILES):
            # prefetch upcoming units
            for u, *_ in segs:
                load_xsh(u)
            nxt = max(u for u, *_ in segs) + 1
            for u2 in (nxt, nxt + 1):
                if u2 < 24:
                    load_xsh(u2)

            cbase = 16 * t

            resp16 = resp_pool.tile([128, L], F16)
            for g in range(2):
                ps = psum_pool.tile([128, 2048], F32, tag="ps")
                for si, (u, klo, khi, rlo, npad) in enumerate(segs):
                    di = u // 4
                    nrow = (khi - klo) + npad
                    if kind == "FT":
                        wcol = di * 192 + (0 if si == 0 else 96)
                    else:
                        wcol = di * 192 + 128
                    lhsT = w_t[:, wcol : wcol + nrow]
                    tp = (0, rlo)
                    for n in range(4):
                        nc.tensor.matmul(
                            ps[rlo : rlo + nrow, n * 512 : (n + 1) * 512],
                            lhsT,
                            xsh[(u, g)][:, n * 512 : (n + 1) * 512],
                            start=True,
                            stop=True,
                            tile_position=tp,
                        )
                nc.scalar.activation(
                    resp16[:, g * 2048 : (g + 1) * 2048], ps[:], act.Copy,
                )

            # sentinel memsets on trim-parity rows (both pad edges at once)
            pstep = resp16[:].ap[0][0]
            for (u, klo, khi, rlo, npad) in segs:
                di = u // 4
                pad = 4 * DILATIONS[di]
                # parity kernels occupy perm positions [0:42); this segment
                # holds perm[klo:khi] at rows rlo..; sentinel rows are the
                # overlap with [0:42).
                plo, phi = max(klo, 0), min(khi, 42)
                if phi <= plo:
                    continue
                row0 = rlo + (plo - klo)
                nrows = phi - plo
                ein = AP(
                    resp16[:].tensor,
                    resp16[:].offset + row0 * pstep,
                    [[pstep, nrows], [L - pad, 2], [1, pad]],
                )
                nc.gpsimd.memset(ein, SENTINEL)

            a_ap = cp_t[:, cbase + 8 : cbase + 12]
            c_ap = cp_t[:, cbase + 12 : cbase + 16]
            b3_ap = cp_t[:, cbase + 3 : cbase + 4]
            nb3_ap = cp_t[:, cbase + 7 : cbase + 8]
            a3h_ap = cp_t[:, cbase + 4 : cbase + 5]  # A3/2 (nbias0 slot)
            boundary = t in (0, NT - 1)
            acc = acc_pool.tile([128, F + 1], F32)
            if boundary:
                # split f0..f2 into column halves: the g0-half runs as soon
                # as the first drain lands (cuts pipeline fill/tail)
                acc2 = acc_pool.tile([128, 3], F32, tag="acc2")
                for f in range(3):
                    b_ap = cp_t[:, cbase + f : cbase + 1 + f]
                    nc.vector.tensor_scalar(
                        trash_d[:, 0:2048], resp16[:, 0:2048], b_ap, None,
                        alu.is_gt, alu.add, accum_out=acc[:, f : f + 1],
                    )
                    nc.vector.tensor_scalar(
                        trash_d[:, 2048:L], resp16[:, 2048:L], b_ap, None,
                        alu.is_gt, alu.add, accum_out=acc2[:, f : f + 1],
                    )
            else:
                for f in range(3):
                    b_ap = cp_t[:, cbase + f : cbase + 1 + f]
                    nc.vector.tensor_scalar(
                        trash_d[:], resp16[:], b_ap, None, alu.is_gt, alu.add,
                        accum_out=acc[:, f : f + 1],
                    )
            # f3: Act counts cols [0:ACT_COLS) as sum-of-sign, DVE the rest
            nc.scalar.activation(
                trash_a[:], resp16[:, 0:ACT_COLS], act.Sign, bias=nb3_ap,
                accum_out=acc[:, 4:5],
            )
            nc.vector.tensor_scalar(
                trash_d[:, ACT_COLS:L], resp16[:, ACT_COLS:L], b3_ap, None,
                alu.is_gt, alu.add, accum_out=acc[:, 3:4],
            )

            u_t = fin_pool.tile([128, F], F32)
            if boundary:
                s_t = fin_pool.tile([128, 3], F32, tag="s_t")
                nc.gpsimd.tensor_add(s_t[:], acc[:, 0:3], acc2[:])
                nc.gpsimd.tensor_mul(u_t[:, 0:3], s_t[:], a_ap[:, 0:3])
                nc.gpsimd.tensor_mul(u_t[:, 3:4], acc[:, 3:4], a_ap[:, 3:4])
            else:
                nc.gpsimd.tensor_mul(u_t[:], acc[:, 0:F], a_ap)
            nc.gpsimd.tensor_sub(fn_global[:, 4 * t : 4 * t + 4], u_t[:], c_ap)
            # fold Act's sign-sum into the staged f3 feature:
            # feat3 += sgn * (A3/2); the +ACT_COLS/2*A3 shift is baked into C3
            s3_t = fin_pool.tile([128, 1], F32, tag="s3_t")
            nc.gpsimd.tensor_mul(s3_t[:], acc[:, 4:5], a3h_ap)
            nc.gpsimd.tensor_add(
                fn_global[:, 4 * t + 3 : 4 * t + 4],
                fn_global[:, 4 * t + 3 : 4 * t + 4], s3_t[:],
            )

        nc.sync.dma_start(AP(out, 0, [[4 * NT, 128], [1, 4 * NT]]), fn_global[:])

    _split_excess_waits(nc)
    return nc


_NC_CACHE = None


def _get_nc():
    global _NC_CACHE
    if _NC_CACHE is None:
        _NC_CACHE = _build_nc()
    return _NC_CACHE


LAST_RESULTS = None


def _host_tables(masks, biasm, mean, std):
    """Build wpack, cpack and the (tile,row)->(unit,kernel) map."""
    perms = [np.array(_perm(di)) for di in range(ND)]

    wpack = np.zeros((ND, 72, 192), np.float16)
    for di in range(ND):
        wt = -masks[di].T[:, perms[di]]  # [C, 84] permuted
        w9 = np.zeros((72, 84), np.float32)
        for j in range(KS):
            w9[j * C : (j + 1) * C, :] = wt
        wpack[di, :, 0:84] = w9
        wpack[di, :, 96:128] = w9[:, 0:32]
        wpack[di, :, 128:180] = w9[:, 32:84]

    mean2 = mean.reshape(ND, K, F)
    std2 = std.reshape(ND, K, F)

    cpack = np.zeros((128, 16 * NT), np.float32)
    rowmap = np.full((NT, 128, 2), -1, np.int64)  # (batch_local, feat_idx)

    for t, (kind, segs) in enumerate(TILES):
        cb = 16 * t
        bias4 = np.full((128, F), 30000.0, np.float32)
        a4 = np.zeros((128, F), np.float32)
        c4 = np.zeros((128, F), np.float32)
        for (u, klo, khi, rlo, npad) in segs:
            di, b = divmod(u, 4)
            d = DILATIONS[di]
            pad = 4 * d
            ks = perms[di][klo:khi]
            rows = np.arange(rlo, rlo + (khi - klo))
            bias4[rows, :] = biasm[di, ks, :]
            par = ((di + ks) % 2 == 1)
            denom = np.where(par, L - 2 * pad, L).astype(np.float32)
            a4[rows, :] = 1.0 / (denom[:, None] * std2[di, ks, :])
            c4[rows, :] = mean2[di, ks, :] / std2[di, ks, :]
            rowmap[t, rows, 0] = b
            rowmap[t, rows, 1] = di * K * F + ks * F  # f=0 index; +f consecutive
        c4[:, 3] = c4[:, 3] - (384 / 2.0) * a4[:, 3]  # ACT_COLS sign shift
        cpack[:, cb : cb + 4] = bias4
        cpack[:, cb + 4 : cb + 8] = -bias4
        cpack[:, cb + 4] = 0.5 * a4[:, 3]  # A3/2 for the sign-count merge
        cpack[:, cb + 8 : cb + 12] = a4
        cpack[:, cb + 12 : cb + 16] = c4

    return wpack, cpack, rowmap


def kernel(x, channel_masks, bias_matrices, feature_mean, feature_std):
    global LAST_RESULTS
    x = np.ascontiguousarray(np.asarray(x, dtype=np.float32))
    masks = np.asarray(channel_masks, dtype=np.float32)
    biasm = np.asarray(bias_matrices, dtype=np.float32)
    mean = np.asarray(feature_mean, dtype=np.float32)
    std = np.asarray(feature_std, dtype=np.float32)

    wpack, cpack, rowmap = _host_tables(masks, biasm, mean, std)

    xt = np.ascontiguousarray(x.transpose(0, 2, 1))
    xp = np.zeros((B, C, LP), np.float16)
    xp[:, :, PADMAX : PADMAX + L] = xt.astype(np.float16)

    nc = _get_nc()
    in_maps = []
    for core in range(N_CORES):
        in_maps.append(
            {
                "xprep": np.ascontiguousarray(xp[core * B_LOC : (core + 1) * B_LOC]),
                "wpack": wpack,
                "cpack": cpack,
            }
        )
    res = run_bass_kernel_spmd(nc, in_maps, list(range(N_CORES)))
    LAST_RESULTS = res

    out = np.zeros((B, NFEAT), np.float32)
    valid = rowmap[:, :, 0] >= 0  # [NT, 128]
    tv, rv = np.nonzero(valid)
    bl = rowmap[tv, rv, 0]  # [n]
    fi = rowmap[tv, rv, 1][:, None] + np.arange(F)[None, :]  # [n, F]
    cols = (tv * F)[:, None] + np.arange(F)[None, :]  # [n, F]
    for core in range(N_CORES):
        res_o = np.asarray(res.results[core]["out"])  # [128, 4*NT]
        out[(core * B_LOC + bl)[:, None], fi] = res_o[rv[:, None], cols]
    return out.astype(np.float32)
